# revision 5
# baseline (speedup 1.0000x reference)
"""Distributed Trainium2 kernel: pixel-shuffle -> W1 linear -> LayerNorm ->
vocab logits -> softmax -> expected token embedding.

Sharding: phase A is data-parallel over batch (core c owns batch c's 256
tokens); an AllGather shares the normalized activations; phase B is tensor
parallel over the vocab (core c owns a 4000-row vocab shard, zero-padded to
4096); ReduceScatter sums the partial numerators/denominators and hands each
core exactly its batch's output slice.

Compute dtype: bf16 matmul inputs with fp32 PSUM accumulation; LayerNorm and
softmax statistics in fp32.
"""

import os
import sys
import types

import numpy as np
import ml_dtypes

N_CORES = 8
B, SEQ, DV = 8, 1024, 1152
DT = 2048          # text hidden size
V = 32000          # vocab
S = 2              # pixel shuffle scale
L = SEQ // (S * S)           # 256 tokens per batch after pixel shuffle
D4 = DV * S * S              # 4608
KA = D4 + 128                # contraction padded: +1 bias row, zero pad to 4736
KT = KA // 128               # 37 k-tiles for phase A
VSH = V // N_CORES           # 4000 true vocab rows per core
VS = 4096                    # padded vocab shard; 32 v-tiles
NT = N_CORES * L             # 2048 total tokens
LN_EPS = 1e-5

LAST_EXEC_TIME_NS = None

_BUILT = None


def _install_ntff_hook_shim():
    """bass_utils' trace path imports antenv.axon_hooks, which is absent in
    this image; provide it via sys.modules using the boot helper."""
    if "antenv.axon_hooks" in sys.modules:
        return
    try:
        from trn_agent_boot.trn_boot import _ntff_profile_via_ctypes

        hook = _ntff_profile_via_ctypes("/opt/axon/libaxon_pjrt.so")
        mod = types.ModuleType("antenv.axon_hooks")
        mod.get_axon_ntff_profile_hook = lambda: hook
        mod.set_axon_ntff_profile_hook = lambda h: None
        sys.modules["antenv.axon_hooks"] = mod
    except Exception:
        pass


def _build():
    import concourse.bass as bass  # noqa: F401
    import concourse.tile as tile
    from concourse import bacc, mybir
    from concourse.masks import make_identity

    f32 = mybir.dt.float32
    bf = mybir.dt.bfloat16
    AF = mybir.ActivationFunctionType
    ALU = mybir.AluOpType

    nc = bacc.Bacc("TRN2", target_bir_lowering=False, debug=False,
                   num_devices=N_CORES)

    xT = nc.declare_dram_parameter("xT", [KA, L], bf, isOutput=False)
    w1T = nc.declare_dram_parameter("w1T", [KA, DT], bf, isOutput=False)
    w2T = nc.declare_dram_parameter("w2T", [DT, VS], bf, isOutput=False)
    emb = nc.declare_dram_parameter("emb", [VS, DT], bf, isOutput=False)
    out = nc.declare_dram_parameter("out", [L, DT], f32, isOutput=True)

    fhat_loc = nc.dram_tensor("fhat_loc", [DT, L], bf)
    fhat_ag = nc.dram_tensor("fhat_ag", [N_CORES, DT, L], bf,
                             addr_space="Shared")
    y_loc = nc.dram_tensor("y_loc", [NT, DT], f32)
    y_rs = nc.dram_tensor("y_rs", [L, DT], f32)
    s_loc = nc.dram_tensor("s_loc", [NT, 1], f32)
    s_rs = nc.dram_tensor("s_rs", [L, 1], f32)

    RG = [list(range(N_CORES))]

    xT_ap = xT.ap().rearrange("(ko p) t -> p ko t", p=128)       # [128,37,256]
    w1T_ap = w1T.ap().rearrange("(ko p) e -> p ko e", p=128)     # [128,37,2048]
    w2T_ap = w2T.ap().rearrange("(ko p) v -> p ko v", p=128)     # [128,16,4096]
    emb_ap = emb.ap().rearrange("(kv p) d -> p kv d", p=128)     # [128,32,2048]
    fhat_loc_ap = fhat_loc.ap().rearrange("(ko p) t -> p ko t", p=128)
    fhat_ag_ap = fhat_ag.ap().rearrange("b (ko p) t -> p ko b t", p=128)

    with tile.TileContext(nc) as tc:
        with tc.tile_pool(name="const", bufs=1) as constp:
            identity = constp.tile([128, 128], bf)
            make_identity(nc, identity)
            eps_t = constp.tile([128, 1], f32)
            nc.vector.memset(eps_t, LN_EPS)
            ones_t = constp.tile([128, 1], bf)
            nc.vector.memset(ones_t, 1.0)
            # last v-tile: partitions >= 32 are vocab padding; bias of -30000
            # before exp flushes them to exactly 0
            mask_t = constp.tile([128, 1], f32)
            nc.vector.memset(mask_t, -30000.0)
            nc.vector.memset(mask_t[0:32, :], 0.0)

            # ------------- phase A: f = pixshuf(x) @ W1^T + b, LayerNorm ----
            with tc.tile_pool(name="sbA", bufs=1) as sbA, \
                 tc.tile_pool(name="w1p", bufs=3) as w1p:
                xT_sb = sbA.tile([128, KT, L], bf)
                nc.sync.dma_start(xT_sb, xT_ap)

                fhat_sb = sbA.tile([128, 2, DT], bf)
                with tc.tile_pool(name="psA", bufs=2, space="PSUM") as psA:
                    pf = [psA.tile([128, 4, 512], f32, tag="pf",
                                   name=f"pf{tt}")
                          for tt in range(2)]
                    for k in range(KT):
                        w1k = w1p.tile([128, DT], bf, tag="w1k")
                        nc.sync.dma_start(w1k, w1T_ap[:, k, :])
                        for tt in range(2):
                            for eb in range(4):
                                nc.tensor.matmul(
                                    pf[tt][:, eb, :],
                                    lhsT=xT_sb[:, k, tt * 128:(tt + 1) * 128],
                                    rhs=w1k[:, eb * 512:(eb + 1) * 512],
                                    start=(k == 0), stop=(k == KT - 1),
                                )
                    # LayerNorm over e (free axis) per token (partition)
                    for tt in range(2):
                        stats = sbA.tile([128, 4, 6], f32, tag="stats")
                        for eb in range(4):
                            nc.vector.bn_stats(out=stats[:, eb, :],
                                               in_=pf[tt][:, eb, :])
                        mv = sbA.tile([128, 2], f32, tag="mv")
                        nc.vector.bn_aggr(out=mv, in_=stats)
                        rstd = sbA.tile([128, 1], f32, tag="rstd")
                        nc.scalar.activation(out=rstd, in_=mv[:, 1:2],
                                             func=AF.Sqrt, bias=eps_t,
                                             scale=1.0)
                        nc.vector.reciprocal(out=rstd, in_=rstd)
                        for eb in range(4):
                            nc.vector.tensor_scalar(
                                out=fhat_sb[:, tt, eb * 512:(eb + 1) * 512],
                                in0=pf[tt][:, eb, :],
                                scalar1=mv[:, 0:1], scalar2=rstd,
                                op0=ALU.subtract, op1=ALU.mult,
                            )
                # transpose fhat [t, e] -> [e, t] via PE
                fhatT_sb = sbA.tile([128, DT // 128, L], bf)
                with tc.tile_pool(name="psT", bufs=2, space="PSUM") as psT:
                    for tt in range(2):
                        for es in range(DT // 128):
                            pt = psT.tile([128, 128], bf, tag="pt")
                            nc.tensor.transpose(
                                pt, fhat_sb[:, tt, es * 128:(es + 1) * 128],
                                identity)
                            nc.any.tensor_copy(
                                out=fhatT_sb[:, es, tt * 128:(tt + 1) * 128],
                                in_=pt)
                nc.sync.dma_start(fhat_loc_ap, fhatT_sb)

            nc.gpsimd.collective_compute(
                "AllGather", mybir.AluOpType.bypass, replica_groups=RG,
                ins=[fhat_loc.ap().opt()], outs=[fhat_ag.ap().opt()])

            # ------------- phase B: logits -> exp -> s, y ------------------
            with tc.tile_pool(name="sbB", bufs=1) as sbB, \
                 tc.tile_pool(name="w2p", bufs=3) as w2p, \
                 tc.tile_pool(name="ep", bufs=2) as epool, \
                 tc.tile_pool(name="yp", bufs=4) as ypool, \
                 tc.tile_pool(name="sp", bufs=2) as spool, \
                 tc.tile_pool(name="psL", bufs=2, space="PSUM") as psL, \
                 tc.tile_pool(name="psY", bufs=2, space="PSUM") as psY, \
                 tc.tile_pool(name="psS", bufs=2, space="PSUM") as psS:
                fhat_all = sbB.tile([128, DT // 128, N_CORES, L], bf)
                for b in range(N_CORES):
                    nc.sync.dma_start(fhat_all[:, :, b, :],
                                      fhat_ag_ap[:, :, b, :])
                fa = fhat_all.rearrange("p ko b t -> p ko (b t)")

                for th in range(2):
                    pbar = sbB.tile([128, VS // 128, 1024], bf, tag="pbar")
                    for v in range(VS // 128):
                        w2t = w2p.tile([128, DT // 128, 128], bf, tag="w2t")
                        nc.sync.dma_start(
                            w2t, w2T_ap[:, :, v * 128:(v + 1) * 128])
                        for tb in range(2):
                            pl = psL.tile([128, 512], f32, tag="pl")
                            t0 = th * 1024 + tb * 512
                            for k2 in range(DT // 128):
                                nc.tensor.matmul(
                                    pl, lhsT=w2t[:, k2, :],
                                    rhs=fa[:, k2, t0:t0 + 512],
                                    start=(k2 == 0),
                                    stop=(k2 == DT // 128 - 1),
                                )
                            nc.scalar.activation(
                                out=pbar[:, v, tb * 512:(tb + 1) * 512],
                                in_=pl, func=AF.Exp,
                                bias=(mask_t if v == VS // 128 - 1 else 0.0),
                                scale=1.0)
                    # softmax denominator partial: s[t] = sum_v pbar[v, t]
                    for sb2 in range(2):
                        ps_ = psS.tile([1, 512], f32, tag="ps")
                        for kv in range(VS // 128):
                            nc.tensor.matmul(
                                ps_, lhsT=ones_t,
                                rhs=pbar[:, kv, sb2 * 512:(sb2 + 1) * 512],
                                start=(kv == 0), stop=(kv == VS // 128 - 1))
                        s_sb = spool.tile([1, 512], f32, tag="s_sb")
                        nc.any.tensor_copy(out=s_sb, in_=ps_)
                        r0 = th * 1024 + sb2 * 512
                        nc.sync.dma_start(
                            s_loc.ap()[r0:r0 + 512, :].rearrange("t o -> o t"),
                            s_sb)
                    # numerator partial: y[t, d] = sum_v pbar[v, t] emb[v, d]
                    for db in range(8):
                        et = epool.tile([128, VS // 128, 256], bf, tag="et")
                        nc.sync.dma_start(
                            et, emb_ap[:, :, db * 256:(db + 1) * 256])
                        for ts2 in range(8):
                            py = psY.tile([128, 256], f32, tag="py")
                            for kv in range(VS // 128):
                                nc.tensor.matmul(
                                    py,
                                    lhsT=pbar[:, kv,
                                              ts2 * 128:(ts2 + 1) * 128],
                                    rhs=et[:, kv, :],
                                    start=(kv == 0),
                                    stop=(kv == VS // 128 - 1))
                            y_sb = ypool.tile([128, 256], f32, tag="y_sb")
                            nc.any.tensor_copy(out=y_sb, in_=py)
                            r0 = th * 1024 + ts2 * 128
                            nc.sync.dma_start(
                                y_loc.ap()[r0:r0 + 128,
                                           db * 256:(db + 1) * 256],
                                y_sb)

            nc.gpsimd.collective_compute(
                "ReduceScatter", mybir.AluOpType.add, replica_groups=RG,
                ins=[y_loc.ap().opt()], outs=[y_rs.ap().opt()])
            nc.gpsimd.collective_compute(
                "ReduceScatter", mybir.AluOpType.add, replica_groups=RG,
                ins=[s_loc.ap().opt()], outs=[s_rs.ap().opt()])

            # ------------- epilogue: out = y_rs / s_rs ---------------------
            with tc.tile_pool(name="fin", bufs=2) as fin:
                for i in range(L // 128):
                    ysb = fin.tile([128, DT], f32, tag="ysb")
                    nc.sync.dma_start(ysb, y_rs.ap()[i * 128:(i + 1) * 128, :])
                    ssb = fin.tile([128, 1], f32, tag="ssb")
                    nc.sync.dma_start(ssb, s_rs.ap()[i * 128:(i + 1) * 128, :])
                    rec = fin.tile([128, 1], f32, tag="rec")
                    nc.vector.reciprocal(out=rec, in_=ssb)
                    osb = fin.tile([128, DT], f32, tag="osb")
                    nc.vector.tensor_scalar_mul(out=osb, in0=ysb, scalar1=rec)
                    nc.sync.dma_start(out.ap()[i * 128:(i + 1) * 128, :], osb)

    nc.finalize()
    return nc


def _pixel_shuffle_np(x, s=S):
    b, seq, d = x.shape
    h = w = int(seq ** 0.5)
    x = x.reshape(b, h, w, d)
    x = x.reshape(b, h, w // s, d * s)
    x = x.transpose(0, 2, 1, 3)
    x = x.reshape(b, w // s, h // s, d * s * s)
    x = x.transpose(0, 2, 1, 3)
    return x.reshape(b, seq // (s * s), d * s * s)


def kernel(vision_feats, llm_token_embed, W1_w, W1_b, W2_w):
    global _BUILT, LAST_EXEC_TIME_NS
    _install_ntff_hook_shim()
    from concourse import bass_utils

    bf16 = ml_dtypes.bfloat16

    if _BUILT is None:
        _BUILT = _build()
    nc = _BUILT

    x = _pixel_shuffle_np(np.asarray(vision_feats, np.float32))  # (8,256,4608)

    w1T_h = np.zeros((KA, DT), bf16)
    w1T_h[:D4] = np.asarray(W1_w, np.float32).T.astype(bf16)
    w1T_h[D4] = np.asarray(W1_b, np.float32).astype(bf16)

    in_maps = []
    for c in range(N_CORES):
        xT_h = np.zeros((KA, L), bf16)
        xT_h[:D4] = x[c].T.astype(bf16)
        xT_h[D4] = 1.0

        w2T_h = np.zeros((DT, VS), bf16)
        w2T_h[:, :VSH] = np.asarray(
            W2_w[c * VSH:(c + 1) * VSH], np.float32).T.astype(bf16)

        emb_h = np.zeros((VS, DT), bf16)
        emb_h[:VSH] = np.asarray(
            llm_token_embed[c * VSH:(c + 1) * VSH], np.float32).astype(bf16)

        in_maps.append({"xT": xT_h, "w1T": w1T_h, "w2T": w2T_h, "emb": emb_h})

    trace = bool(os.environ.get("KERNEL_TRACE"))
    kwargs = {}
    if trace:
        kwargs["trace"] = True
        kwargs["tmpdir"] = os.environ.get("KERNEL_TRACE_DIR") or None

    res = bass_utils.run_bass_kernel_spmd(
        nc, in_maps, core_ids=list(range(N_CORES)), **kwargs)
    LAST_EXEC_TIME_NS = res.exec_time_ns

    out_full = np.stack(
        [np.asarray(res.results[c]["out"]) for c in range(N_CORES)], axis=0)
    return out_full.astype(np.float32)


# revision 7
# speedup vs baseline: 1.0423x; 1.0423x over previous
"""Distributed Trainium2 kernel: pixel-shuffle -> W1 linear -> LayerNorm ->
vocab logits -> softmax -> expected token embedding.

Sharding: phase A is data-parallel over batch (core c owns batch c's 256
tokens); an AllGather (chunked per 128-token half) shares the normalized
activations; phase B is tensor parallel over the vocab (core c owns a
4000-row vocab shard, zero-padded to 4096); per-half ReduceScatters sum the
partial numerators/denominators and hand each core exactly its batch's
output slice, overlapping the first half's collective with the second
half's compute.

Token halves are interleaved by batch: half h = every batch's local tokens
[128h, 128h+128), so a ReduceScatter chunk r within a half is exactly batch
r's 128-token block.

Compute dtype: bf16 matmul inputs with fp32 PSUM accumulation; LayerNorm
and softmax statistics in fp32.
"""

import os
import sys
import types

import numpy as np
import ml_dtypes

N_CORES = 8
B, SEQ, DV = 8, 1024, 1152
DT = 2048          # text hidden size
V = 32000          # vocab
S = 2              # pixel shuffle scale
L = SEQ // (S * S)           # 256 tokens per batch after pixel shuffle
D4 = DV * S * S              # 4608
KA = D4 + 128                # contraction padded: +1 bias row, zero pad to 4736
KT = KA // 128               # 37 k-tiles for phase A
VSH = V // N_CORES           # 4000 true vocab rows per core
VS = 4096                    # padded vocab shard; 32 v-tiles
VT = VS // 128               # 32
ET = DT // 128               # 16
LN_EPS = 1e-5

LAST_EXEC_TIME_NS = None

_BUILT = None


def _install_ntff_hook_shim():
    """bass_utils' trace path imports antenv.axon_hooks, which is absent in
    this image; provide it via sys.modules using the boot helper."""
    if "antenv.axon_hooks" in sys.modules:
        return
    try:
        from trn_agent_boot.trn_boot import _ntff_profile_via_ctypes

        hook = _ntff_profile_via_ctypes("/opt/axon/libaxon_pjrt.so")
        mod = types.ModuleType("antenv.axon_hooks")
        mod.get_axon_ntff_profile_hook = lambda: hook
        mod.set_axon_ntff_profile_hook = lambda h: None
        sys.modules["antenv.axon_hooks"] = mod
    except Exception:
        pass


def _build():
    import concourse.bass as bass  # noqa: F401
    import concourse.tile as tile
    from concourse import bacc, mybir
    from concourse.masks import make_identity

    f32 = mybir.dt.float32
    bf = mybir.dt.bfloat16
    AF = mybir.ActivationFunctionType
    ALU = mybir.AluOpType

    nc = bacc.Bacc("TRN2", target_bir_lowering=False, debug=False,
                   num_devices=N_CORES)

    xT = nc.declare_dram_parameter("xT", [KA, L], bf, isOutput=False)
    w1T = nc.declare_dram_parameter("w1T", [KA, DT], bf, isOutput=False)
    w2T = nc.declare_dram_parameter("w2T", [DT, VS], bf, isOutput=False)
    emb = nc.declare_dram_parameter("emb", [VS, DT], bf, isOutput=False)
    out = nc.declare_dram_parameter("out", [L, DT], f32, isOutput=True)

    fhat_loc = [nc.dram_tensor(f"fhat_loc{h}", [DT, 128], bf)
                for h in range(2)]
    fhat_ag = [nc.dram_tensor(f"fhat_ag{h}", [N_CORES, DT, 128], bf,
                              addr_space="Shared") for h in range(2)]
    y_loc = [nc.dram_tensor(f"y_loc{h}", [N_CORES * 128, DT], f32)
             for h in range(2)]
    y_rs = [nc.dram_tensor(f"y_rs{h}", [128, DT], f32) for h in range(2)]
    s_loc = [nc.dram_tensor(f"s_loc{h}", [N_CORES * 128, 1], f32)
             for h in range(2)]
    s_rs = [nc.dram_tensor(f"s_rs{h}", [128, 1], f32) for h in range(2)]

    RG = [list(range(N_CORES))]

    xT_ap = xT.ap().rearrange("(ko p) t -> p ko t", p=128)       # [128,37,256]
    w1T_ap = w1T.ap().rearrange("(ko p) e -> p ko e", p=128)     # [128,37,2048]
    w2T_ap = w2T.ap().rearrange("(ko p) v -> p ko v", p=128)     # [128,16,4096]
    emb_ap = emb.ap().rearrange("(kv p) d -> p kv d", p=128)     # [128,32,2048]
    fhat_loc_ap = [t.ap().rearrange("(ko p) t -> p ko t", p=128)
                   for t in fhat_loc]
    fhat_ag_ap = [t.ap().rearrange("b (ko p) t -> p ko b t", p=128)
                  for t in fhat_ag]

    with tile.TileContext(nc) as tc:
        with tc.tile_pool(name="const", bufs=1) as constp:
            identity = constp.tile([128, 128], bf)
            make_identity(nc, identity)
            eps_t = constp.tile([128, 1], f32)
            nc.vector.memset(eps_t, LN_EPS)
            ones_t = constp.tile([128, 1], bf)
            nc.vector.memset(ones_t, 1.0)
            # last v-tile: partitions >= 32 are vocab padding; bias of -30000
            # before exp flushes them to exactly 0
            mask_t = constp.tile([128, 1], f32)
            nc.vector.memset(mask_t, -30000.0)
            nc.vector.memset(mask_t[0:32, :], 0.0)

            # PE warmup: ~4us of junk matmuls so HAM unthrottles before the
            # DMA-fed phase A matmuls arrive
            with tc.tile_pool(name="psW", bufs=1, space="PSUM") as psW:
                junk = psW.tile([128, 128], f32)
                for _ in range(36):
                    nc.tensor.matmul(junk, lhsT=identity, rhs=identity,
                                     start=True, stop=True,
                                     skip_group_check=True)

            # ------------- phase A: f = pixshuf(x) @ W1^T + b, LayerNorm ----
            with tc.tile_pool(name="sbA", bufs=1) as sbA, \
                 tc.tile_pool(name="w1p", bufs=8) as w1p:
                xT_sb = sbA.tile([128, KT, L], bf)
                nc.sync.dma_start(xT_sb, xT_ap)

                fhat_sb = sbA.tile([128, 2, DT], bf)
                with tc.tile_pool(name="psA", bufs=2, space="PSUM") as psA:
                    pf = [psA.tile([128, 4, 512], f32, tag="pf",
                                   name=f"pf{tt}")
                          for tt in range(2)]
                    for k in range(KT):
                        w1k = w1p.tile([128, DT], bf, tag="w1k")
                        nc.sync.dma_start(w1k, w1T_ap[:, k, :])
                        for tt in range(2):
                            for eb in range(4):
                                nc.tensor.matmul(
                                    pf[tt][:, eb, :],
                                    lhsT=xT_sb[:, k, tt * 128:(tt + 1) * 128],
                                    rhs=w1k[:, eb * 512:(eb + 1) * 512],
                                    start=(k == 0), stop=(k == KT - 1),
                                )
                    # LayerNorm over e (free axis) per token (partition)
                    for tt in range(2):
                        stats = sbA.tile([128, 4, 6], f32, tag="stats")
                        for eb in range(4):
                            nc.vector.bn_stats(out=stats[:, eb, :],
                                               in_=pf[tt][:, eb, :])
                        mv = sbA.tile([128, 2], f32, tag="mv")
                        nc.vector.bn_aggr(out=mv, in_=stats)
                        rstd = sbA.tile([128, 1], f32, tag="rstd")
                        nc.scalar.activation(out=rstd, in_=mv[:, 1:2],
                                             func=AF.Sqrt, bias=eps_t,
                                             scale=1.0)
                        nc.vector.reciprocal(out=rstd, in_=rstd)
                        for eb in range(4):
                            nc.vector.tensor_scalar(
                                out=fhat_sb[:, tt, eb * 512:(eb + 1) * 512],
                                in0=pf[tt][:, eb, :],
                                scalar1=mv[:, 0:1], scalar2=rstd,
                                op0=ALU.subtract, op1=ALU.mult,
                            )
                # transpose fhat [t, e] -> [e, t] via PE; AllGather per half
                with tc.tile_pool(name="psT", bufs=2, space="PSUM") as psT:
                    for tt in range(2):
                        fhatT_sb = sbA.tile([128, ET, 128], bf, tag="fhatT",
                                            name=f"fhatT{tt}")
                        for es in range(ET):
                            pt = psT.tile([128, 128], bf, tag="pt")
                            nc.tensor.transpose(
                                pt, fhat_sb[:, tt, es * 128:(es + 1) * 128],
                                identity)
                            nc.any.tensor_copy(
                                out=fhatT_sb[:, es, :], in_=pt)
                        nc.sync.dma_start(fhat_loc_ap[tt], fhatT_sb)
                        nc.gpsimd.collective_compute(
                            "AllGather", mybir.AluOpType.bypass,
                            replica_groups=RG,
                            ins=[fhat_loc[tt].ap().opt()],
                            outs=[fhat_ag[tt].ap().opt()])

            # ------------- phase B: logits -> exp -> s, y ------------------
            with tc.tile_pool(name="sbB", bufs=2) as sbB, \
                 tc.tile_pool(name="pbp", bufs=1) as pbp, \
                 tc.tile_pool(name="w2p", bufs=3) as w2p, \
                 tc.tile_pool(name="ep", bufs=2) as epool, \
                 tc.tile_pool(name="yp", bufs=4) as ypool, \
                 tc.tile_pool(name="sp", bufs=2) as spool, \
                 tc.tile_pool(name="psL", bufs=2, space="PSUM") as psL, \
                 tc.tile_pool(name="psY", bufs=2, space="PSUM") as psY, \
                 tc.tile_pool(name="psS", bufs=2, space="PSUM") as psS:
                for th in range(2):
                    # [128, e-tile, batch, tok] activations for this half
                    fa = sbB.tile([128, ET, N_CORES, 128], bf, tag="fa",
                                  name=f"fa{th}")
                    for b in range(N_CORES):
                        nc.sync.dma_start(fa[:, :, b, :],
                                          fhat_ag_ap[th][:, :, b, :])
                    pbar = pbp.tile([128, VT, 1024], bf, tag="pbar")
                    for v in range(VT):
                        w2t = w2p.tile([128, ET, 128], bf, tag="w2t")
                        nc.sync.dma_start(
                            w2t, w2T_ap[:, :, v * 128:(v + 1) * 128])
                        for tb in range(2):
                            pl = psL.tile([128, 512], f32, tag="pl")
                            for k2 in range(ET):
                                nc.tensor.matmul(
                                    pl, lhsT=w2t[:, k2, :],
                                    rhs=fa[:, k2, tb * 4:(tb + 1) * 4, :],
                                    start=(k2 == 0),
                                    stop=(k2 == ET - 1),
                                )
                            nc.scalar.activation(
                                out=pbar[:, v, tb * 512:(tb + 1) * 512],
                                in_=pl, func=AF.Exp,
                                bias=(mask_t if v == VT - 1 else 0.0),
                                scale=1.0)
                    # softmax denominator partial: s[t] = sum_v pbar[v, t]
                    for sb2 in range(2):
                        ps_ = psS.tile([1, 512], f32, tag="ps")
                        for kv in range(VT):
                            nc.tensor.matmul(
                                ps_, lhsT=ones_t,
                                rhs=pbar[:, kv, sb2 * 512:(sb2 + 1) * 512],
                                start=(kv == 0), stop=(kv == VT - 1))
                        s_sb = spool.tile([1, 512], f32, tag="s_sb")
                        nc.any.tensor_copy(out=s_sb, in_=ps_)
                        r0 = sb2 * 512
                        nc.sync.dma_start(
                            s_loc[th].ap()[r0:r0 + 512, :]
                            .rearrange("t o -> o t"),
                            s_sb)
                    # numerator partial: y[t, d] = sum_v pbar[v, t] emb[v, d]
                    for db in range(8):
                        et = epool.tile([128, VT, 256], bf, tag="et")
                        nc.sync.dma_start(
                            et, emb_ap[:, :, db * 256:(db + 1) * 256])
                        for ts2 in range(8):
                            py = psY.tile([128, 256], f32, tag="py")
                            for kv in range(VT):
                                nc.tensor.matmul(
                                    py,
                                    lhsT=pbar[:, kv,
                                              ts2 * 128:(ts2 + 1) * 128],
                                    rhs=et[:, kv, :],
                                    start=(kv == 0),
                                    stop=(kv == VT - 1))
                            y_sb = ypool.tile([128, 256], f32, tag="y_sb")
                            nc.any.tensor_copy(out=y_sb, in_=py)
                            nc.sync.dma_start(
                                y_loc[th].ap()[ts2 * 128:(ts2 + 1) * 128,
                                               db * 256:(db + 1) * 256],
                                y_sb)
                    nc.gpsimd.collective_compute(
                        "ReduceScatter", mybir.AluOpType.add,
                        replica_groups=RG,
                        ins=[y_loc[th].ap().opt()],
                        outs=[y_rs[th].ap().opt()])
                    nc.gpsimd.collective_compute(
                        "ReduceScatter", mybir.AluOpType.add,
                        replica_groups=RG,
                        ins=[s_loc[th].ap().opt()],
                        outs=[s_rs[th].ap().opt()])

            # ------------- epilogue: out = y_rs / s_rs ---------------------
            with tc.tile_pool(name="fin", bufs=2) as fin:
                for th in range(2):
                    ysb = fin.tile([128, DT], f32, tag="ysb")
                    nc.sync.dma_start(ysb, y_rs[th].ap()[:, :])
                    ssb = fin.tile([128, 1], f32, tag="ssb")
                    nc.sync.dma_start(ssb, s_rs[th].ap()[:, :])
                    rec = fin.tile([128, 1], f32, tag="rec")
                    nc.vector.reciprocal(out=rec, in_=ssb)
                    osb = fin.tile([128, DT], f32, tag="osb")
                    nc.vector.tensor_scalar_mul(out=osb, in0=ysb, scalar1=rec)
                    nc.sync.dma_start(out.ap()[th * 128:(th + 1) * 128, :],
                                      osb)

    nc.finalize()
    return nc


def _pixel_shuffle_np(x, s=S):
    b, seq, d = x.shape
    h = w = int(seq ** 0.5)
    x = x.reshape(b, h, w, d)
    x = x.reshape(b, h, w // s, d * s)
    x = x.transpose(0, 2, 1, 3)
    x = x.reshape(b, w // s, h // s, d * s * s)
    x = x.transpose(0, 2, 1, 3)
    return x.reshape(b, seq // (s * s), d * s * s)


def kernel(vision_feats, llm_token_embed, W1_w, W1_b, W2_w):
    global _BUILT, LAST_EXEC_TIME_NS
    _install_ntff_hook_shim()
    from concourse import bass_utils

    bf16 = ml_dtypes.bfloat16

    if _BUILT is None:
        _BUILT = _build()
    nc = _BUILT

    x = _pixel_shuffle_np(np.asarray(vision_feats, np.float32))  # (8,256,4608)

    w1T_h = np.zeros((KA, DT), bf16)
    w1T_h[:D4] = np.asarray(W1_w, np.float32).T.astype(bf16)
    w1T_h[D4] = np.asarray(W1_b, np.float32).astype(bf16)

    in_maps = []
    for c in range(N_CORES):
        xT_h = np.zeros((KA, L), bf16)
        xT_h[:D4] = x[c].T.astype(bf16)
        xT_h[D4] = 1.0

        w2T_h = np.zeros((DT, VS), bf16)
        w2T_h[:, :VSH] = np.asarray(
            W2_w[c * VSH:(c + 1) * VSH], np.float32).T.astype(bf16)

        emb_h = np.zeros((VS, DT), bf16)
        emb_h[:VSH] = np.asarray(
            llm_token_embed[c * VSH:(c + 1) * VSH], np.float32).astype(bf16)

        in_maps.append({"xT": xT_h, "w1T": w1T_h, "w2T": w2T_h, "emb": emb_h})

    trace = bool(os.environ.get("KERNEL_TRACE"))
    kwargs = {}
    if trace:
        import tempfile

        kwargs["trace"] = True
        base = os.environ.get("KERNEL_TRACE_DIR")
        if base:
            os.makedirs(base, exist_ok=True)
            kwargs["tmpdir"] = tempfile.mkdtemp(dir=base)
        print("trace dir:", kwargs.get("tmpdir"), file=sys.stderr)

    res = bass_utils.run_bass_kernel_spmd(
        nc, in_maps, core_ids=list(range(N_CORES)), **kwargs)
    LAST_EXEC_TIME_NS = res.exec_time_ns

    # out rows are each batch's tokens in order (half 0 = rows 0:128,
    # half 1 = rows 128:256) -- matches the natural token order
    out_full = np.stack(
        [np.asarray(res.results[c]["out"]) for c in range(N_CORES)], axis=0)
    return out_full.astype(np.float32)
